# revision 3
# baseline (speedup 1.0000x reference)
"""Trainium2 Bass kernel for nn_BertEventSGNN (gnn_message_passing).

Computation (per graph b of 256, K=40 event nodes, H=128):
  1. ev[b,k,:] = seq_out[r, tgt[r], :]   with r = b*64 + nid2rows[b, ev_nids[b,k], 0]
  2. a[b]     = adj[b]^T @ ev[b] + adj_bias
  3. h = GRU cell applied twice: h1 = gru(a, ev), h2 = gru(h1, h1); return h2

Sharding: data-parallel over graphs, 32 graphs per core across 8 cores.
Each core receives its seq_out shard (flattened to [32*64*128, 128] token
rows) and gathers only the 1280 rows it needs via indirect DMA (512 B per
row) instead of streaming the full shard.  The GRU runs in transposed
layout [H, rows] so the gate weights are PE-stationary operands, the gate
biases ride the ScalarE activation bias port for free, and no per-pass
transposes are needed.  adj^T @ ev is computed as ev^T-contract matmuls
with 3 graphs packed block-diagonally into one 128-partition PE tile.

Host-side work is limited to index arithmetic (gather row ids) and layout
prep of the small parameter tensors (transpose weights, fold adj_bias into
the pass-1 GRU biases, build the block-diagonal adj tiles).
"""

import sys

sys.path.insert(0, "/opt/trn_rl_repo")

import numpy as np

import concourse.bass as bass
import concourse.bacc as bacc
import concourse.mybir as mybir
from concourse.bass_utils import run_bass_kernel_spmd
from concourse.tile import TileContext

B = 256
N_INST = 64
L = 128
H = 128
K = 40
NCORES = 8
BG = B // NCORES          # graphs per core = 32
ROWS = BG * K             # GRU rows per core = 1280
GPG = 3                   # graphs per block-diagonal group
NG = (BG + GPG - 1) // GPG  # groups per core = 11
GW = GPG * K              # group width = 120

F32 = mybir.dt.float32
I32 = mybir.dt.int32
AF = mybir.ActivationFunctionType
OP = mybir.AluOpType

_NC_CACHE = {}


def _build_nc():
    nc = bacc.Bacc()
    seq = nc.declare_dram_parameter("seq", [BG * N_INST * L, H], F32, isOutput=False)
    gidx = nc.declare_dram_parameter("gidx", [128, NG], I32, isOutput=False)
    adjbd = nc.declare_dram_parameter("adjbd", [NG, 128, GW], F32, isOutput=False)
    wih = nc.declare_dram_parameter("wih", [H, 3 * H], F32, isOutput=False)
    whh = nc.declare_dram_parameter("whh", [H, 3 * H], F32, isOutput=False)
    w2 = nc.declare_dram_parameter("w2", [H, 2 * H], F32, isOutput=False)
    bias = nc.declare_dram_parameter("bias", [H, 8], F32, isOutput=False)
    identd = nc.declare_dram_parameter("ident", [128, 128], F32, isOutput=False)
    out = nc.declare_dram_parameter("out", [ROWS, H], F32, isOutput=True)

    with TileContext(nc) as tc:
        with (
            tc.tile_pool(name="const", bufs=1) as cpool,
            tc.tile_pool(name="grp", bufs=3) as gpool,
            tc.tile_pool(name="work", bufs=2) as wpool,
            tc.tile_pool(name="state", bufs=1) as spool,
            tc.tile_pool(name="psg", bufs=2, space="PSUM") as psg,
            tc.tile_pool(name="pgate", bufs=1, space="PSUM") as pgate,
        ):
            ident = cpool.tile([128, 128], F32)
            nc.gpsimd.dma_start(out=ident[:], in_=identd[:, :])
            wih_sb = cpool.tile([H, 3 * H], F32)
            nc.gpsimd.dma_start(out=wih_sb[:], in_=wih[:, :])
            whh_sb = cpool.tile([H, 3 * H], F32)
            nc.gpsimd.dma_start(out=whh_sb[:], in_=whh[:, :])
            w2_sb = cpool.tile([H, 2 * H], F32)
            nc.gpsimd.dma_start(out=w2_sb[:], in_=w2[:, :])
            b_sb = cpool.tile([H, 8], F32)
            nc.gpsimd.dma_start(out=b_sb[:], in_=bias[:, :])
            idx_sb = cpool.tile([128, NG], I32)
            nc.gpsimd.dma_start(out=idx_sb[:], in_=gidx[:, :])

            xT = spool.tile([128, ROWS], F32)    # (adj^T @ ev)^T, bias folded into GRU biases
            h0T = spool.tile([128, ROWS], F32)   # ev^T
            h1T = spool.tile([128, ROWS], F32)
            h2T = spool.tile([128, ROWS], F32)

            # Phase 1: gather ev rows, adj message passing, build xT / h0T.
            for g in range(NG):
                w = min(BG - g * GPG, GPG) * K
                eg = gpool.tile([128, 128], F32, tag="eg")
                nc.gpsimd.indirect_dma_start(
                    out=eg[:, :],
                    out_offset=None,
                    in_=seq[:, :],
                    in_offset=bass.IndirectOffsetOnAxis(ap=idx_sb[:, g : g + 1], axis=0),
                )
                ag = gpool.tile([128, GW], F32, tag="ag")
                nc.gpsimd.dma_start(out=ag[:, :], in_=adjbd[g, :, :])
                ps_a = psg.tile([128, GW], F32, tag="ps_a")
                nc.tensor.matmul(
                    ps_a[:, :w], lhsT=eg[:, :], rhs=ag[:, :w], start=True, stop=True
                )
                nc.vector.tensor_copy(out=xT[:, g * GW : g * GW + w], in_=ps_a[:, :w])
                ps_t = psg.tile([128, 128], F32, tag="ps_t")
                nc.tensor.transpose(ps_t[:, :], eg[:, :], ident[:])
                nc.vector.tensor_copy(out=h0T[:, g * GW : g * GW + w], in_=ps_t[:, :w])

            # Phase 2: two GRU passes in transposed layout.
            chunks = [(0, 512), (512, 512), (1024, 256)]
            for p, (xs, hs, hd, bo) in enumerate(
                [(xT, h0T, h1T, 0), (h1T, h1T, h2T, 4)]
            ):
                for c0, cw in chunks:
                    xc = xs[:, c0 : c0 + cw]
                    hc = hs[:, c0 : c0 + cw]
                    ps_r = pgate.tile([128, 512], F32, tag="ps_r")
                    ps_z = pgate.tile([128, 512], F32, tag="ps_z")
                    ps_i = pgate.tile([128, 512], F32, tag="ps_i")
                    ps_n = pgate.tile([128, 512], F32, tag="ps_n")
                    if p == 0:
                        nc.tensor.matmul(ps_r[:, :cw], lhsT=wih_sb[:, 0:128], rhs=xc, start=True, stop=False)
                        nc.tensor.matmul(ps_r[:, :cw], lhsT=whh_sb[:, 0:128], rhs=hc, start=False, stop=True)
                        nc.tensor.matmul(ps_z[:, :cw], lhsT=wih_sb[:, 128:256], rhs=xc, start=True, stop=False)
                        nc.tensor.matmul(ps_z[:, :cw], lhsT=whh_sb[:, 128:256], rhs=hc, start=False, stop=True)
                        nc.tensor.matmul(ps_i[:, :cw], lhsT=wih_sb[:, 256:384], rhs=xc, start=True, stop=True)
                        nc.tensor.matmul(ps_n[:, :cw], lhsT=whh_sb[:, 256:384], rhs=hc, start=True, stop=True)
                    else:
                        # x == h, so r/z contract with pre-summed weights.
                        nc.tensor.matmul(ps_r[:, :cw], lhsT=w2_sb[:, 0:128], rhs=hc, start=True, stop=True)
                        nc.tensor.matmul(ps_z[:, :cw], lhsT=w2_sb[:, 128:256], rhs=hc, start=True, stop=True)
                        nc.tensor.matmul(ps_i[:, :cw], lhsT=wih_sb[:, 256:384], rhs=hc, start=True, stop=True)
                        nc.tensor.matmul(ps_n[:, :cw], lhsT=whh_sb[:, 256:384], rhs=hc, start=True, stop=True)
                    r = wpool.tile([128, 512], F32, tag="r")
                    nc.scalar.activation(r[:, :cw], ps_r[:, :cw], AF.Sigmoid, bias=b_sb[:, bo : bo + 1])
                    z = wpool.tile([128, 512], F32, tag="z")
                    nc.scalar.activation(z[:, :cw], ps_z[:, :cw], AF.Sigmoid, bias=b_sb[:, bo + 1 : bo + 2])
                    nt = wpool.tile([128, 512], F32, tag="nt")
                    nc.vector.scalar_tensor_tensor(
                        out=nt[:, :cw], in0=ps_n[:, :cw], scalar=b_sb[:, bo + 3 : bo + 4],
                        in1=r[:, :cw], op0=OP.add, op1=OP.mult,
                    )
                    s = wpool.tile([128, 512], F32, tag="s")
                    nc.vector.tensor_tensor(out=s[:, :cw], in0=nt[:, :cw], in1=ps_i[:, :cw], op=OP.add)
                    n = wpool.tile([128, 512], F32, tag="n")
                    nc.scalar.activation(n[:, :cw], s[:, :cw], AF.Tanh, bias=b_sb[:, bo + 2 : bo + 3])
                    c_ = wpool.tile([128, 512], F32, tag="c")
                    nc.vector.tensor_tensor(out=c_[:, :cw], in0=hc, in1=n[:, :cw], op=OP.subtract)
                    t_ = wpool.tile([128, 512], F32, tag="t")
                    nc.vector.tensor_tensor(out=t_[:, :cw], in0=c_[:, :cw], in1=z[:, :cw], op=OP.mult)
                    nc.vector.tensor_tensor(out=hd[:, c0 : c0 + cw], in0=t_[:, :cw], in1=n[:, :cw], op=OP.add)

            # Phase 3: transpose h2 back to [rows, H] and store.
            for c in range(ROWS // 128):
                ps_o = psg.tile([128, 128], F32, tag="ps_a")
                nc.tensor.transpose(ps_o[:, :], h2T[:, c * 128 : (c + 1) * 128], ident[:])
                ot = wpool.tile([128, 128], F32, tag="ot")
                nc.vector.tensor_copy(out=ot[:, :], in_=ps_o[:, :])
                nc.sync.dma_start(out=out[c * 128 : (c + 1) * 128, :], in_=ot[:, :])
    nc.compile()
    return nc


def get_nc():
    if "nc" not in _NC_CACHE:
        _NC_CACHE["nc"] = _build_nc()
    return _NC_CACHE["nc"]


def prep_in_maps(inputs):
    seq_out = np.ascontiguousarray(np.asarray(inputs["seq_out"]), dtype=np.float32)
    tgt = np.asarray(inputs["target_idxs"]).astype(np.int64).reshape(B, N_INST)
    nid = np.asarray(inputs["nid2rows"]).astype(np.int64)[:, :, 0]
    ev = np.asarray(inputs["ev_nids"]).astype(np.int64)
    adj = np.asarray(inputs["adj"]).astype(np.float32)
    adj_bias = np.asarray(inputs["adj_bias"]).astype(np.float32)
    w_ih = np.asarray(inputs["w_ih"]).astype(np.float32)
    w_hh = np.asarray(inputs["w_hh"]).astype(np.float32)
    b_ih = np.asarray(inputs["b_ih"]).astype(np.float32)
    b_hh = np.asarray(inputs["b_hh"]).astype(np.float32)

    inst_of_ev = np.take_along_axis(nid, ev, axis=1)               # [B, K]
    tok_of_ev = np.take_along_axis(tgt, inst_of_ev, axis=1)        # [B, K]
    local_b = np.arange(B) % BG
    row_idx = (local_b[:, None] * N_INST + inst_of_ev) * L + tok_of_ev
    rows_core = row_idx.reshape(NCORES, BG * K).astype(np.int32)

    gidx3 = np.zeros((NCORES, NG, 128), np.int32)
    for g in range(NG):
        j0 = g * GW
        j1 = min(j0 + GW, BG * K)
        gidx3[:, g, : j1 - j0] = rows_core[:, j0:j1]
    gidx_t = np.ascontiguousarray(gidx3.transpose(0, 2, 1))        # [NCORES, 128, NG]

    adjc = adj.reshape(NCORES, BG, K, K)
    adjbd = np.zeros((NCORES, NG, 128, GW), np.float32)
    for g in range(NG):
        for t in range(GPG):
            b = g * GPG + t
            if b < BG:
                adjbd[:, g, t * K : (t + 1) * K, t * K : (t + 1) * K] = adjc[:, b]

    wihT = np.ascontiguousarray(w_ih.T)                            # [H, 3H]
    whhT = np.ascontiguousarray(w_hh.T)
    w2T = np.ascontiguousarray((w_ih[: 2 * H] + w_hh[: 2 * H]).T)  # [H, 2H]
    cvec = w_ih @ adj_bias                                          # adj_bias folded into pass-1 biases
    bias = np.zeros((H, 8), np.float32)
    bias[:, 0] = b_ih[0:H] + b_hh[0:H] + cvec[0:H]
    bias[:, 1] = b_ih[H : 2 * H] + b_hh[H : 2 * H] + cvec[H : 2 * H]
    bias[:, 2] = b_ih[2 * H : 3 * H] + cvec[2 * H : 3 * H]
    bias[:, 3] = b_hh[2 * H : 3 * H]
    bias[:, 4] = b_ih[0:H] + b_hh[0:H]
    bias[:, 5] = b_ih[H : 2 * H] + b_hh[H : 2 * H]
    bias[:, 6] = b_ih[2 * H : 3 * H]
    bias[:, 7] = b_hh[2 * H : 3 * H]

    seq_sh = seq_out.reshape(NCORES, BG * N_INST * L, H)
    return [
        {
            "seq": seq_sh[c],
            "gidx": gidx_t[c],
            "adjbd": adjbd[c],
            "wih": wihT,
            "whh": whhT,
            "w2": w2T,
            "bias": bias,
            "ident": np.eye(128, dtype=np.float32),
        }
        for c in range(NCORES)
    ]


def run(inputs, trace=False, **kwargs):
    in_maps = prep_in_maps(inputs)
    nc = get_nc()
    res = run_bass_kernel_spmd(nc, in_maps, list(range(NCORES)), trace=trace, **kwargs)
    out = np.concatenate([res.results[c]["out"] for c in range(NCORES)], axis=0)
    return out.reshape(B, K, H), res


def kernel(**inputs):
    out, _ = run(inputs, trace=False)
    return out
